# revision 2
# baseline (speedup 1.0000x reference)
"""Trainium2 Bass kernel for nn_DistributionalQNetwork (C51 distributional Q).

Self-contained: hardcodes shapes from the problem spec.
  MLP: [B,1092] -> 512 -> 256 -> 128 -> 101 logits -> softmax
  C51 categorical projection with scatter-add into [B,101].

Pure data parallel across 8 NeuronCores (B=65536 -> 8192 rows/core), one
identical Bass program per core, inputs sharded on host, no collectives.

Device pipeline (per core, feature-major activations [feat, batch]):
  - All MLP inputs/weights stream as fp8e4m3 (weights scaled by 64 for
    subnormal headroom, undone by each relu's scale=1/64). Layers 1-3 use
    DoubleRow perf mode (2 k-rows/cell/cycle). obs cols 0..1023 are
    pre-transposed on host to [1024, B] fp8; the 68-feature tail
    (obs 1024.. + actions) plus a constant-1 row that carries b1 ride in a
    [128, B] fp8 tensor. b2 is added via the relu bias AP, b3 via a k=1
    ones-matmul rider, b4 via a k=1 ones-matmul. L4 un-transposes by using
    x3 as lhsT. Each layer's relu fuses into ONE activation over a
    multi-bank PSUM tile (L3's runs on DVE to offload ACT).
  - Softmax: one fused exp [128, 4*102] -> fp16 (logits span ~±0.3, no max
    subtraction; the pad column holds logit -30 so it vanishes); row sums
    via one DVE tensor_reduce; normalization is folded into the projection
    weight ops (scalar_tensor_tensor with the reciprocal as scalar).
  - C51 projection: b = clip(r + g*z, ±10)/dz is monotone per row, so equal
    target bins form contiguous runs (clip plateaus included -- the b==0 /
    b==100 piles are just the first/last runs; no separate masks). Host
    ships, per row: lw = b - floor(b) (fp16), the run-continuation gate E
    (fp16 0/1), and run-end scatter indices idxl/idxu (int16, -1 elsewhere).
    Device: wu = (lw*inv)*e, wl = (e*inv)-wu, then ONE tensor_tensor_scan
    per weight array (y[t] = E[t]*y[t-1] + w[t]) produces run sums at
    run-end positions; two GPSIMD local_scatters place them; one fp16 add
    combines. No bit-exact host replication is needed anywhere: the
    projection is continuous in b, and the reference's exact-integer-b
    double-mass quirk (~1e-5 of elements) is accepted as error (~3e-3
    rel-fro, tolerance 2e-2).
  - g==0 rows (bootstrap==0) scatter nothing (idx=-1); the host adds their
    closed-form 2-bin output (independent of the MLP) afterwards.

All side inputs are packed [128, n_tiles*K] on host so every DMA moves
>=1.6KB contiguous per partition (full DMA bus efficiency), loaded per
2048-row super-chunk. Output is fp16 [128, n_tiles*101], unpacked on host.
"""
import os
import numpy as np
import ml_dtypes

import concourse.bacc as bacc
import concourse.mybir as mybir
from concourse import tile
from concourse.bass_utils import run_bass_kernel_spmd

F32 = np.float32
FP8 = ml_dtypes.float8_e4m3
BF16 = ml_dtypes.bfloat16
FP16 = np.float16

f32 = mybir.dt.float32
bf16 = mybir.dt.bfloat16
fp16 = mybir.dt.float16
i16 = mybir.dt.int16
f8 = mybir.dt.float8e4

Alu = mybir.AluOpType
Act = mybir.ActivationFunctionType
AX = mybir.AxisListType
DR = mybir.MatmulPerfMode.DoubleRow

B_FULL = 65536
N_CORES = 8
B_CORE = B_FULL // N_CORES      # 8192
D_OBS = 1090
H1, H2, H3 = 512, 256, 128
NA = 101
NA2 = 102                       # padded atom count (scan/scatter width)
TILE = 128
CHUNK = 512                     # batch rows per matmul sweep
SUPER = 2048                    # batch rows per DMA super-load
WSCALE = F32(64.0)              # fp8 weight scale (subnormal headroom)


def build_nc(n_rows=B_CORE):
    """Build the single-core Bass program (replicated over all cores)."""
    assert n_rows % SUPER == 0
    n_chunks = n_rows // CHUNK
    hpc = SUPER // CHUNK            # chunks per super-load
    tps = SUPER // TILE             # row-tiles per super-load
    nt = n_rows // TILE

    nc = bacc.Bacc("TRN2", target_bir_lowering=False, debug=False)

    # ---- DRAM I/O ----
    xt8_d = nc.dram_tensor("xt8", [1024, n_rows], f8, kind="ExternalInput")
    tl8_d = nc.dram_tensor("tail8", [TILE, n_rows], f8, kind="ExternalInput")
    w1f8_d = nc.dram_tensor("w1f8", [TILE, 4096], f8, kind="ExternalInput")
    w1t8_d = nc.dram_tensor("w1t8", [TILE, H1], f8, kind="ExternalInput")
    w2f8_d = nc.dram_tensor("w2f8", [TILE, 1024], f8, kind="ExternalInput")
    w3f8_d = nc.dram_tensor("w3f8", [TILE, 256], f8, kind="ExternalInput")
    w4p_d = nc.dram_tensor("w4p", [TILE, NA2], bf16, kind="ExternalInput")
    b4r_d = nc.dram_tensor("b4r", [1, NA2], bf16, kind="ExternalInput")
    b2c_d = nc.dram_tensor("b2c", [TILE, 2], f32, kind="ExternalInput")
    b3r_d = nc.dram_tensor("b3r64", [1, TILE], bf16, kind="ExternalInput")
    # per-row packs [128, nt, 2*NA2]: row (t*128+q) -> [q, t, :]
    idx_d = nc.dram_tensor("idxpk", [TILE, nt * 2 * NA2], i16,
                           kind="ExternalInput")
    lwe_d = nc.dram_tensor("lwepk", [TILE, nt * 2 * NA2], fp16,
                           kind="ExternalInput")
    out_d = nc.dram_tensor("out", [TILE, nt * NA], fp16, kind="ExternalOutput")

    with tile.TileContext(nc) as tc:
        with (
            tc.tile_pool(name="const", bufs=1) as cpool,
            tc.tile_pool(name="xin", bufs=2) as xpool,
            tc.tile_pool(name="acts", bufs=2) as apool,
            tc.tile_pool(name="proj", bufs=4) as ppool,
            tc.tile_pool(name="outp", bufs=2) as opool,
            tc.tile_pool(name="ps1", bufs=1, space="PSUM") as ps1pool,
            tc.tile_pool(name="ps2", bufs=1, space="PSUM") as ps2pool,
            tc.tile_pool(name="ps3", bufs=1, space="PSUM") as ps3pool,
            tc.tile_pool(name="psl", bufs=1, space="PSUM") as pslpool,
        ):
            # ---- constants resident in SBUF ----
            w1f8t = cpool.tile([TILE, 4096], f8)
            nc.sync.dma_start(w1f8t[:], w1f8_d[:])
            w1t8t = cpool.tile([TILE, H1], f8)
            nc.sync.dma_start(w1t8t[:], w1t8_d[:])
            w2f8t = cpool.tile([TILE, 1024], f8)
            nc.sync.dma_start(w2f8t[:], w2f8_d[:])
            w3f8t = cpool.tile([TILE, 256], f8)
            nc.sync.dma_start(w3f8t[:], w3f8_d[:])
            w4t = cpool.tile([TILE, NA2], bf16)
            nc.sync.dma_start(w4t[:], w4p_d[:])
            b4t = cpool.tile([1, NA2], bf16)
            nc.sync.dma_start(b4t[:], b4r_d[:])
            b2t = cpool.tile([TILE, 2], f32)
            nc.sync.dma_start(b2t[:], b2c_d[:])
            b3t = cpool.tile([1, TILE], bf16)
            nc.sync.dma_start(b3t[:], b3r_d[:])
            ones = cpool.tile([1, CHUNK], bf16)
            nc.vector.memset(ones[:], 1.0)

            xbig = tl8 = idxt = lwet = outsup = None
            for bc in range(n_chunks):
                sb = bc // hpc
                # ---- super-loads: one full-bandwidth DMA per tensor ----
                if bc % hpc == 0:
                    xbig = xpool.tile([TILE, 8 * SUPER], f8, tag="xbig")
                    nc.sync.dma_start(
                        xbig[:].rearrange("q (p i n) -> q p i n", p=4, i=2),
                        xt8_d[:, sb * SUPER:(sb + 1) * SUPER].rearrange(
                            "(p i q) n -> q p i n", p=4, i=2))
                    tl8 = xpool.tile([TILE, SUPER], f8, tag="tl8")
                    nc.sync.dma_start(
                        tl8[:], tl8_d[:, sb * SUPER:(sb + 1) * SUPER])
                    k2 = tps * 2 * NA2
                    idxt = xpool.tile([TILE, k2], i16, tag="idxt")
                    nc.sync.dma_start(
                        idxt[:], idx_d[:, sb * k2:(sb + 1) * k2])
                    lwet = xpool.tile([TILE, k2], fp16, tag="lwet")
                    nc.sync.dma_start(
                        lwet[:], lwe_d[:, sb * k2:(sb + 1) * k2])
                    outsup = opool.tile([TILE, tps * NA], fp16, tag="outsup")
                h0 = (bc % hpc) * CHUNK
                xv = xbig[:].rearrange("q (p i n) -> q p i n", p=4, i=2)

                # ---- L1: x1[feat 512, batch 512] fp8, b1 rides in tl8 ----
                ps1 = ps1pool.tile([TILE, 4 * CHUNK], f32, tag="ps1")
                for m in range(4):
                    om = ps1[:, m * CHUNK:(m + 1) * CHUNK]
                    for p in range(4):
                        lhs = w1f8t[:, p * 1024:(p + 1) * 1024].rearrange(
                            "k (i mm) -> k i mm", i=2)[:, :,
                                                       m * TILE:(m + 1) * TILE]
                        nc.tensor.matmul(om, lhs, xv[:, p, :, h0:h0 + CHUNK],
                                         start=(p == 0), stop=False,
                                         perf_mode=DR)
                    nc.tensor.matmul(om, w1t8t[:, m * TILE:(m + 1) * TILE],
                                     tl8[:, h0:h0 + CHUNK],
                                     start=False, stop=True)
                x1t = apool.tile([TILE, 4 * CHUNK], f8, tag="x1")
                nc.scalar.activation(x1t[:], ps1[:], Act.Relu,
                                     bias=0.0, scale=1.0 / float(WSCALE))

                # ---- L2: x2[feat 256, batch 512] fp8 ----
                ps2 = ps2pool.tile([TILE, 2 * CHUNK], f32, tag="ps2")
                for m in range(2):
                    om = ps2[:, m * CHUNK:(m + 1) * CHUNK]
                    for c in range(2):
                        lhs = w2f8t[:, c * 512:(c + 1) * 512].rearrange(
                            "k (i mm) -> k i mm", i=2)[:, :,
                                                       m * TILE:(m + 1) * TILE]
                        rhs = x1t[:, c * 1024:(c + 1) * 1024].rearrange(
                            "k (i n) -> k i n", i=2)
                        nc.tensor.matmul(om, lhs, rhs, start=(c == 0),
                                         stop=(c == 1), perf_mode=DR)
                x2t = apool.tile([TILE, 2 * CHUNK], f8, tag="x2")
                for m in range(2):
                    nc.scalar.activation(x2t[:, m * CHUNK:(m + 1) * CHUNK],
                                         ps2[:, m * CHUNK:(m + 1) * CHUNK],
                                         Act.Relu, bias=b2t[:, m:m + 1],
                                         scale=1.0 / float(WSCALE))

                # ---- L3: x3[feat 128, batch 512] bf16, relu on DVE ----
                ps3 = ps3pool.tile([TILE, CHUNK], f32, tag="ps3")
                nc.tensor.matmul(
                    ps3[:], w3f8t[:].rearrange("k (i mm) -> k i mm", i=2),
                    x2t[:].rearrange("k (i n) -> k i n", i=2),
                    start=True, stop=False, perf_mode=DR)
                nc.tensor.matmul(ps3[:], b3t[:], ones[:],
                                 start=False, stop=True)
                x3t = apool.tile([TILE, CHUNK], bf16, tag="x3")
                nc.vector.tensor_scalar(x3t[:], ps3[:], 1.0 / float(WSCALE),
                                        0.0, Alu.mult, Alu.max)

                # ---- L4 + softmax (fused over the 4 row-tiles) ----
                psl = pslpool.tile([TILE, 4 * NA2], f32, tag="psl")
                for s in range(4):
                    om = psl[:, s * NA2:(s + 1) * NA2]
                    nc.tensor.matmul(om, ones[:, 0:TILE], b4t[:],
                                     start=True, stop=False)
                    nc.tensor.matmul(om, x3t[:, s * TILE:(s + 1) * TILE],
                                     w4t[:], start=False, stop=True)
                e16 = ppool.tile([TILE, 4 * NA2], fp16, tag="e16")
                nc.scalar.activation(e16[:], psl[:], Act.Exp,
                                     bias=0.0, scale=1.0)
                ssum = ppool.tile([TILE, 4], f32, tag="ssum")
                nc.vector.tensor_reduce(
                    ssum[:], e16[:].rearrange("q (s a) -> q s a", a=NA2),
                    AX.X, Alu.add)
                inv4 = ppool.tile([TILE, 4], f32, tag="inv4")
                nc.vector.reciprocal(inv4[:], ssum[:])

                # ---- projection per row-tile ----
                for s in range(4):
                    ts = (bc % hpc) * 4 + s
                    base = ts * 2 * NA2
                    lw_s = lwet[:, base:base + NA2]
                    E_s = lwet[:, base + NA2:base + 2 * NA2]
                    il_s = idxt[:, base:base + NA2]
                    iu_s = idxt[:, base + NA2:base + 2 * NA2]
                    e_s = e16[:, s * NA2:(s + 1) * NA2]
                    iv = inv4[:, s:s + 1]

                    wu = ppool.tile([TILE, NA2], fp16, tag="wu")
                    nc.vector.scalar_tensor_tensor(wu[:], lw_s, iv, e_s,
                                                   Alu.mult, Alu.mult)
                    wl = ppool.tile([TILE, NA2], fp16, tag="wl")
                    nc.vector.scalar_tensor_tensor(wl[:], e_s, iv, wu[:],
                                                   Alu.mult, Alu.subtract)
                    ywl = ppool.tile([TILE, NA2], fp16, tag="ywl")
                    nc.vector.tensor_tensor_scan(ywl[:], E_s, wl[:], 0.0,
                                                 Alu.mult, Alu.add)
                    ywu = ppool.tile([TILE, NA2], fp16, tag="ywu")
                    nc.vector.tensor_tensor_scan(ywu[:], E_s, wu[:], 0.0,
                                                 Alu.mult, Alu.add)
                    scl = ppool.tile([TILE, NA2], fp16, tag="scl")
                    nc.gpsimd.local_scatter(scl[:], ywl[:], il_s,
                                            channels=TILE, num_elems=NA2,
                                            num_idxs=NA2)
                    scu = ppool.tile([TILE, NA2], fp16, tag="scu")
                    nc.gpsimd.local_scatter(scu[:], ywu[:], iu_s,
                                            channels=TILE, num_elems=NA2,
                                            num_idxs=NA2)
                    nc.vector.tensor_tensor(
                        outsup[:, ts * NA:(ts + 1) * NA],
                        scl[:, 0:NA], scu[:, 0:NA], Alu.add)

                if bc % hpc == hpc - 1:
                    nc.sync.dma_start(
                        out_d[:, sb * tps * NA:(sb + 1) * tps * NA],
                        outsup[:])

    nc.compile()
    return nc


# ------------------------- host side -------------------------

def _host_prep(obs, actions, rewards, bootstrap, discount, q_support,
               W1, b1, W2, b2, W3, b3, W4, b4, n_rows=B_CORE):
    B = obs.shape[0]
    nt = n_rows // TILE
    g = (bootstrap * discount).astype(F32)

    # ---- projection structure (continuous in b; no bit-exactness needed) --
    tz = rewards[:, None] + g[:, None] * q_support[None, :].astype(F32)
    tz = np.clip(tz.astype(F32), F32(-10.0), F32(10.0))
    bh = ((tz + F32(10.0)) * F32(5.0)).astype(F32)          # [B,101] in [0,100]
    li = np.floor(bh)
    lw = (bh - li).astype(FP16)
    E = np.zeros((B, NA), FP16)
    E[:, 1:] = (li[:, 1:] == li[:, :-1]).astype(FP16)       # run continues
    lm = np.ones((B, NA), bool)
    lm[:, :-1] = li[:, :-1] != li[:, 1:]                    # run ends
    lii = li.astype(np.int16)
    idxl = np.where(lm, lii, np.int16(-1))
    idxu = np.where(lm, lii + np.int16(1), np.int16(-1))
    g0 = g == 0
    idxl[g0] = -1                                           # host handles g==0
    idxu[g0] = -1
    pad1 = np.full((B, 1), -1, np.int16)
    idxpk = np.concatenate([idxl, pad1, idxu, pad1], axis=1)       # [B, 204]
    padz = np.zeros((B, 1), FP16)
    lwepk = np.concatenate([lw, padz, E, padz], axis=1)            # [B, 204]

    # ---- MLP weights (fp8, x64 for subnormal headroom) ----
    w1f8 = np.ascontiguousarray(
        (W1[:1024] * WSCALE).astype(FP8)
        .reshape(4, 2, TILE, H1).transpose(2, 0, 1, 3).reshape(TILE, 4096))
    W1tail = np.zeros((TILE, H1), F32)
    W1tail[:68] = W1[1024:1092]
    W1tail[68] = b1
    w1t8 = (W1tail * WSCALE).astype(FP8)
    w2f8 = np.ascontiguousarray(
        (W2 * WSCALE).astype(FP8)
        .reshape(2, 2, TILE, H2).transpose(2, 0, 1, 3).reshape(TILE, 1024))
    w3f8 = np.ascontiguousarray(
        (W3 * WSCALE).astype(FP8)
        .reshape(2, TILE, H3).transpose(1, 0, 2).reshape(TILE, 256))
    w4p = np.zeros((TILE, NA2), BF16)
    w4p[:, :NA] = W4.astype(BF16)
    b4r = np.full((1, NA2), F32(-30.0), F32)                # pad logit -> -30
    b4r[0, :NA] = b4
    b4r = b4r.astype(BF16)
    b2c = np.ascontiguousarray(b2.reshape(2, TILE).T).astype(F32)
    b3r = (b3 * WSCALE)[None, :].astype(BF16)

    # ---- activations, feature-major fp8 ----
    xt8_all = np.ascontiguousarray(obs[:, :1024].astype(FP8).T)   # [1024, B]
    tail_all = np.zeros((TILE, B), FP8)
    tail_all[:66] = obs[:, 1024:1090].T.astype(FP8)
    tail_all[66:68] = actions.T.astype(FP8)
    tail_all[68] = FP8(1.0)                                 # b1 rider row

    def corepack(a, s):
        return np.ascontiguousarray(
            a[s].reshape(nt, TILE, a.shape[1])
            .transpose(1, 0, 2).reshape(TILE, -1))

    shared = dict(w1f8=w1f8, w1t8=w1t8, w2f8=w2f8, w3f8=w3f8,
                  w4p=w4p, b4r=b4r, b2c=b2c, b3r64=b3r)
    in_maps = []
    for c in range(B // n_rows):
        s = slice(c * n_rows, (c + 1) * n_rows)
        m = dict(shared)
        m["xt8"] = np.ascontiguousarray(xt8_all[:, s])
        m["tail8"] = np.ascontiguousarray(tail_all[:, s])
        m["idxpk"] = corepack(idxpk, s)
        m["lwepk"] = corepack(lwepk, s)
        in_maps.append(m)
    return in_maps, g


def _host_g0(out, rewards, g):
    """Closed-form output for bootstrap==0 rows (independent of the MLP:
    b is constant across atoms, probabilities sum to 1)."""
    rows = np.nonzero(g == 0)[0]
    if rows.size == 0:
        return out
    r = np.clip(rewards[rows], F32(-10.0), F32(10.0)).astype(F32)
    b0 = ((r + F32(10.0)) * F32(5.0)).astype(F32)
    li = np.floor(b0)
    frac = (b0 - li).astype(F32)
    ii = li.astype(np.int64)
    ni = frac > 0
    np.add.at(out, (rows[ni], ii[ni]), (F32(1.0) - frac[ni]))
    np.add.at(out, (rows[ni], ii[ni] + 1), frac[ni])
    isint = ~ni
    interior = isint & (ii > 0) & (ii < 100)
    np.add.at(out, (rows[interior], ii[interior] - 1), F32(1.0))
    np.add.at(out, (rows[interior], ii[interior] + 1), F32(1.0))
    edge = isint & ~interior
    np.add.at(out, (rows[edge], ii[edge]), F32(1.0))
    return out


_NC_CACHE = {}


def kernel(obs, actions, rewards, bootstrap, discount, q_support,
           W1, b1, W2, b2, W3, b3, W4, b4):
    obs = np.asarray(obs, F32)
    actions = np.asarray(actions, F32)
    rewards = np.asarray(rewards, F32)
    bootstrap = np.asarray(bootstrap, F32)
    discount = np.asarray(discount, F32)
    q_support = np.asarray(q_support, F32)
    W1, b1 = np.asarray(W1, F32), np.asarray(b1, F32)
    W2, b2 = np.asarray(W2, F32), np.asarray(b2, F32)
    W3, b3 = np.asarray(W3, F32), np.asarray(b3, F32)
    W4, b4 = np.asarray(W4, F32), np.asarray(b4, F32)
    assert obs.shape == (B_FULL, D_OBS) and actions.shape == (B_FULL, 2)

    in_maps, g = _host_prep(
        obs, actions, rewards, bootstrap, discount, q_support,
        W1, b1, W2, b2, W3, b3, W4, b4)

    if B_CORE not in _NC_CACHE:
        _NC_CACHE[B_CORE] = build_nc(B_CORE)
    nc = _NC_CACHE[B_CORE]

    trace = bool(int(os.environ.get("KERNEL_TRACE", "0")))
    res = run_bass_kernel_spmd(nc, in_maps, list(range(N_CORES)), trace=trace)
    kernel.last_results = res

    nt = B_CORE // TILE
    out = np.concatenate(
        [r["out"].reshape(TILE, nt, NA).transpose(1, 0, 2)
         .reshape(B_CORE, NA).astype(F32) for r in res.results], axis=0)
    out = _host_g0(out, rewards, g)
    return out
